# revision 1
# baseline (speedup 1.0000x reference)
"""Trainium2 Bass kernel for the GatedODEFlow problem.

Math: the reference iterates  a <- a + h*alpha(a) * (tgt - a)  where
alpha depends on a only through the low-rank projection (a - mu) @ U / S.
Since each step is a per-row convex blend toward the fixed vector tgt,
a_t = c_t * x + (1 - c_t) * tgt  for a per-row scalar c_t, and the
projection evolves affinely in c_t:

    proj_t = c_t * (x@W - tgt@W) + (tgt@W - mu@W)   with W = U / (S+1e-6)
    dist2_t = A * c_t^2 + B2 * c_t + C              (per-row A, B2; global C)
    alpha_t = exp(-dist2_t / (2*k*sigma^2))
    c_{t+1} = c_t * (1 - h * alpha_t),  c_0 = 1
    out = c_N * x + (1 - c_N) * tgt

So the device only needs ONE matmul q0 = x @ W per row plus a scalar
recurrence and a final fused blend: read x once, write out once
(memory-bound roofline).

Implementation notes:
- The projection contracts over the d dimension, so x must be transposed
  on-chip (PE transpose). We transpose the stride-2 uint16 view of x
  (the high half of an fp32 IS its truncated bf16), which makes the
  transposes single-pass and feeds the projection matmul in bf16 with no
  cast instructions at all. The gate only determines the scalar alpha,
  so bf16 there costs ~3e-4 relative output error.
- The blend out = c*x + (1-c)*tgt reads the original fp32 x: most chunks
  use a PE rank-1 outer product (1-c)x(tgt) accumulated in PSUM plus one
  fused DVE scalar_tensor_tensor; a slice of chunks instead goes through
  the otherwise-idle GPSIMD engine using a replicated-target SBUF tile.
- DMA: one 8 MiB load and one 8 MiB store per 512-row macroblock
  (batched for bandwidth), on separate HWDGE queues (SP for loads, ACT
  for stores) so loads never queue behind stores.
- Emission is software-pipelined: macro m's back-half (d-transpose,
  outer products, blend) is emitted after macro m+1's PE-heavy front so
  the in-order PE stream never waits on the serial gate recurrence.

Sharding: data-parallel across 8 cores along the batch dim; small
parameters replicated (per the problem's sharding hint).
"""

import math
import os
from contextlib import ExitStack

import numpy as np
import ml_dtypes

import concourse.bass as bass
import concourse.mybir as mybir
import concourse.tile as tile
from concourse import bacc
from concourse.masks import make_identity
from concourse.bass_utils import run_bass_kernel_spmd

F32 = mybir.dt.float32
BF16 = mybir.dt.bfloat16
I16 = mybir.dt.int16
AF = mybir.ActivationFunctionType
OP = mybir.AluOpType

N_CORES = 8
D = 4096
KSUB = 64
SUB = 128            # rows per subblock (one partition tile)
SPM = 4              # subblocks per macroblock
MACRO = SUB * SPM    # 512 rows
DCH = 128            # d-chunk width for PE transposes
NDCH = D // DCH      # 32
CCH = 512            # combine chunk width
NCCH = D // CCH      # 8
GP_CHUNKS = 0        # blend chunks per subblock routed to GPSIMD (of NCCH)

_PROGRAM_CACHE: dict = {}
LAST_RESULT = None


def _build_program(rows: int, num_steps: int, neg_inv: float, exp_bias: float,
                   neg_h: float):
    nmacro = rows // MACRO
    assert rows == nmacro * MACRO, f"rows {rows} not a multiple of {MACRO}"

    nc = bacc.Bacc("TRN2")
    x_d = nc.dram_tensor("x", [rows, D], F32, kind="ExternalInput")
    w_d = nc.dram_tensor("w", [D, KSUB], BF16, kind="ExternalInput")
    tgt_d = nc.dram_tensor("tgt", [1, D], BF16, kind="ExternalInput")
    tgf_d = nc.dram_tensor("tgf", [1, D], F32, kind="ExternalInput")
    nqt_d = nc.dram_tensor("nqt", [KSUB, 1], F32, kind="ExternalInput")
    abr_d = nc.dram_tensor("abr", [128, 2], BF16, kind="ExternalInput")
    out_d = nc.dram_tensor("out", [rows, D], F32, kind="ExternalOutput")

    with ExitStack() as ctx:
        tc = ctx.enter_context(tile.TileContext(nc))
        singles = ctx.enter_context(tc.tile_pool(name="singles", bufs=1))
        xpool = ctx.enter_context(tc.tile_pool(name="xp", bufs=10))
        xtpool = ctx.enter_context(tc.tile_pool(name="xtp", bufs=4))
        stkpool = ctx.enter_context(tc.tile_pool(name="stkp", bufs=2))
        smpool = ctx.enter_context(tc.tile_pool(name="smp", bufs=2))
        ptr = ctx.enter_context(tc.tile_pool(name="ptr", bufs=2, space="PSUM"))
        pq = ctx.enter_context(tc.tile_pool(name="pq", bufs=1, space="PSUM"))
        pab = ctx.enter_context(tc.tile_pool(name="pab", bufs=1, space="PSUM"))
        pdt = ctx.enter_context(tc.tile_pool(name="pdt", bufs=1, space="PSUM"))
        pout = ctx.enter_context(tc.tile_pool(name="pout", bufs=3, space="PSUM"))

        ident32 = singles.tile([128, 128], F32)
        make_identity(nc, ident32)
        identu = singles.tile([128, 128], BF16)
        make_identity(nc, identu)
        w_sb = singles.tile([128, NDCH, KSUB], BF16)
        nc.sync.dma_start(out=w_sb, in_=w_d[:, :].rearrange("(j p) k -> p j k", p=128))
        tgt_sb = singles.tile([1, D], BF16)
        nc.sync.dma_start(out=tgt_sb, in_=tgt_d[:, :])
        nqt_sb = singles.tile([KSUB, 1], F32)
        nc.sync.dma_start(out=nqt_sb, in_=nqt_d[:, :])
        abr_sb = singles.tile([128, 2], BF16)
        nc.sync.dma_start(out=abr_sb, in_=abr_d[:, :])
        ebias_sb = singles.tile([128, 1], F32)
        nc.vector.memset(ebias_sb, exp_bias)

        def emit_front(m):
            """Load + PE transposes + bf16 projection + extraction + A/B."""
            r0 = m * MACRO
            xs = []
            xus = []
            for s in range(SPM):
                xin = xpool.tile([SUB, D], F32, tag="xin")
                nc.sync.dma_start(
                    out=xin, in_=x_d[r0 + s * SUB : r0 + (s + 1) * SUB, :])
                xs.append(xin)
                # stride-2 bf16 view: element 2d+1 is the high half of
                # x[d], i.e. the truncated-bf16 value of x[d]
                xus.append(xin.bitcast(BF16).rearrange(
                    "p (d two) -> p d two", two=2))

            q0T = pq.tile([KSUB, MACRO], F32, tag="q0T")
            for j in range(NDCH):
                tp = ptr.tile([128, MACRO], BF16, tag="tp")
                for s in range(SPM):
                    nc.tensor.transpose(
                        tp[:, s * SUB : (s + 1) * SUB],
                        xus[s][:, j * DCH : (j + 1) * DCH, 1], identu)
                xt = xtpool.tile([128, MACRO], BF16, tag="xt")
                nc.scalar.copy(xt, tp)
                nc.tensor.matmul(
                    q0T, w_sb[:, j, :], xt,
                    start=(j == 0), stop=(j == NDCH - 1))

            # stk rows 0..63 = (q0T - qT)^2 ; rows 64..127 = (q0T - qT)
            stk = stkpool.tile([128, MACRO], BF16, tag="stk")
            nc.scalar.activation(stk[0:KSUB, :], q0T, AF.Square,
                                 bias=nqt_sb, scale=1.0)
            nc.scalar.activation(stk[KSUB:128, :], q0T, AF.Identity,
                                 bias=nqt_sb, scale=1.0)
            ab = pab.tile([128, 2 * SPM], F32, tag="ab")
            for s in range(SPM):
                lhs = stk[:, s * SUB : (s + 1) * SUB]
                nc.tensor.matmul(ab[:, s : s + 1], lhs,
                                 abr_sb[:, 0:1], start=True, stop=True)
                nc.tensor.matmul(ab[:, SPM + s : SPM + s + 1], lhs,
                                 abr_sb[:, 1:2], start=True, stop=True)
            return {"xs": xs, "ab": ab, "r0": r0}

        def emit_iteration(st):
            """Per-row scalar recurrence (DVE + ACT exp) -> c, d."""
            ab = st["ab"]
            A = ab[:, 0:SPM]
            B2 = ab[:, SPM : 2 * SPM]
            c = smpool.tile([128, SPM], F32, tag="c")
            nc.vector.memset(c, 1.0)
            t1 = smpool.tile([128, SPM], F32, tag="t1")
            alpha = smpool.tile([128, SPM], F32, tag="alpha")
            for _t in range(num_steps):
                nc.vector.tensor_tensor(t1, A, c, OP.mult)
                nc.vector.tensor_tensor(t1, t1, B2, OP.add)
                nc.vector.tensor_tensor(t1, t1, c, OP.mult)
                nc.scalar.activation(alpha, t1, AF.Exp,
                                     bias=ebias_sb, scale=neg_inv)
                nc.vector.tensor_tensor(t1, alpha, c, OP.mult)
                nc.vector.scalar_tensor_tensor(c, t1, neg_h, c, OP.mult, OP.add)
            d_t = smpool.tile([128, SPM], BF16, tag="d")
            nc.vector.tensor_scalar(d_t, c, -1.0, 1.0, OP.mult, OP.add)
            st["c"] = c
            st["d_t"] = d_t

        def emit_drows(st):
            """PE-transpose d into per-subblock rows (input ready by now)."""
            dT = pdt.tile([1, MACRO], BF16, tag="dT")
            drows = []
            for s in range(SPM):
                nc.tensor.transpose(dT[:, s * SUB : (s + 1) * SUB],
                                    st["d_t"][:, s : s + 1], identu)
                dr = smpool.tile([1, SUB], BF16, tag=f"dr{s}")
                nc.vector.tensor_copy(dr, dT[:, s * SUB : (s + 1) * SUB])
                drows.append(dr)
            st["drows"] = drows

        def emit_blend_store(st):
            """out = c*x + (1-c)*tgt in place over x, then store."""
            xs, c, drows, r0 = st["xs"], st["c"], st["drows"], st["r0"]
            for s in range(SPM):
                cs = c[:, s : s + 1]
                for h2 in range(NCCH):
                    xsl = xs[s][:, h2 * CCH : (h2 + 1) * CCH]
                    op_ps = pout.tile([128, CCH], F32, tag="op")
                    nc.tensor.matmul(
                        op_ps, drows[s],
                        tgt_sb[:, h2 * CCH : (h2 + 1) * CCH],
                        start=True, stop=True)
                    nc.vector.scalar_tensor_tensor(
                        xsl, xsl, cs, op_ps, OP.mult, OP.add)
                nc.scalar.dma_start(
                    out=out_d[r0 + s * SUB : r0 + (s + 1) * SUB, :],
                    in_=xs[s])

        # Software-pipelined emission (see module docstring).
        prev = None
        for m in range(nmacro):
            st = emit_front(m)
            if prev is not None:
                emit_drows(prev)
                emit_blend_store(prev)
            emit_iteration(st)
            prev = st
        emit_drows(prev)
        emit_blend_store(prev)

    if not nc.is_finalized():
        nc.finalize()
    return nc


def _get_program(rows, num_steps, neg_inv, exp_bias, neg_h):
    key = (rows, num_steps, neg_inv, exp_bias, neg_h)
    if key not in _PROGRAM_CACHE:
        _PROGRAM_CACHE[key] = _build_program(rows, num_steps, neg_inv,
                                             exp_bias, neg_h)
    return _PROGRAM_CACHE[key]


def kernel(x, manifold_mu, manifold_U, manifold_S, attractor_mu,
           log_step, sigma, num_steps):
    global LAST_RESULT
    x = np.ascontiguousarray(np.asarray(x, dtype=np.float32))
    mu = np.asarray(manifold_mu, dtype=np.float64)
    U = np.asarray(manifold_U, dtype=np.float64)
    S = np.asarray(manifold_S, dtype=np.float64)
    tgt = np.asarray(attractor_mu, dtype=np.float64)
    ls = float(np.asarray(log_step))
    sg = float(np.asarray(sigma))
    ns = int(np.asarray(num_steps))

    batch, dmodel = x.shape
    assert dmodel == D and batch % N_CORES == 0

    if ns <= 0:
        return x.copy()

    # Host-side parameter folding (O(D*K), trivial). qT/qmu/C use the
    # truncated-bf16 W so they are consistent with the device projection,
    # which feeds bf16(x)~truncation and bf16(W) into the matmul.
    W = U / (S + 1e-6)[None, :]
    W16 = W.astype(ml_dtypes.bfloat16)
    Wq = W16.astype(np.float64)
    qT = tgt @ Wq
    qmu = mu @ Wq
    wt = qT - qmu
    Cc = float(wt @ wt)
    inv = 1.0 / (float(KSUB) * 2.0 * sg * sg * 1.0)  # TEMPERATURE = 1.0
    step = min(max(math.exp(ls), 1e-3), 1.0)
    h = step / ns

    neg_inv = -inv
    exp_bias = -inv * Cc
    neg_h = -h

    rows = batch // N_CORES
    nc = _get_program(rows, ns, neg_inv, exp_bias, neg_h)

    abr = np.zeros((128, 2), ml_dtypes.bfloat16)
    abr[0:KSUB, 0] = 1.0
    abr[KSUB:128, 1] = (2.0 * wt).astype(ml_dtypes.bfloat16)
    tgt32 = np.ascontiguousarray(tgt.astype(np.float32)[None, :])
    common = {
        "w": np.ascontiguousarray(W16),
        "tgt": np.ascontiguousarray(tgt.astype(ml_dtypes.bfloat16)[None, :]),
        "tgf": tgt32,
        "nqt": np.ascontiguousarray((-qT).astype(np.float32)[:, None]),
        "abr": abr,
    }
    in_maps = [
        {"x": x[i * rows : (i + 1) * rows], **common} for i in range(N_CORES)
    ]

    trace = bool(int(os.environ.get("GOF_TRACE", "0")))
    res = run_bass_kernel_spmd(nc, in_maps, list(range(N_CORES)), trace=trace)
    LAST_RESULT = res
    out = np.concatenate([res.results[i]["out"] for i in range(N_CORES)],
                         axis=0)
    return out



# revision 2
# speedup vs baseline: 1.0762x; 1.0762x over previous
"""Trainium2 Bass kernel for the GatedODEFlow problem.

Math: the reference iterates  a <- a + h*alpha(a) * (tgt - a)  where
alpha depends on a only through the low-rank projection (a - mu) @ U / S.
Since each step is a per-row convex blend toward the fixed vector tgt,
a_t = c_t * x + (1 - c_t) * tgt  for a per-row scalar c_t, and the
projection evolves affinely in c_t:

    proj_t = c_t * (x@W - tgt@W) + (tgt@W - mu@W)   with W = U / (S+1e-6)
    dist2_t = A * c_t^2 + B2 * c_t + C              (per-row A, B2; global C)
    alpha_t = exp(-dist2_t / (2*k*sigma^2))
    c_{t+1} = c_t * (1 - h * alpha_t),  c_0 = 1
    out = c_N * x + (1 - c_N) * tgt

So the device only needs ONE matmul q0 = x @ W per row plus a scalar
recurrence and a final fused blend: read x once (fp32), write out once
(fp16 -- the 2e-2 rel-err budget dwarfs fp16 rounding, and halving the
store stream moves the DMA roofline from 375us to ~270us per core).

v2 engine layout (per 512-row macroblock, knobs at module top):
- PE: 128 transposes of the stride-2 bf16 view of x (the high half of an
  fp32 IS its truncated bf16), 32 projection matmuls, 4 small A/B
  matmuls.  No more rank-1 outer products: the blend term (1-c)*tgt is
  built from a host-replicated target tile instead.
- ACT: PSUM->SBUF copies of most transposed groups, the gate Square/
  Identity extraction, exp, c*x for the GPSIMD blend chunks
  (per-partition `scale=` AP), store dma_starts.
- DVE: remaining copies, ttmp = (1-c)*tgt per subblock (4x-mode bf16
  tensor_scalar with per-partition scalar), the scalar recurrence, and
  the blend STT out = c*x + ttmp for chunks 0..NCCH-GP_CHUNKS-1.
- GPSIMD: blend adds (out = xc + ttmp) for the trailing GP_CHUNKS
  chunks; otherwise idle in the baseline.
- DMA: one 8 MiB fp32 load + one 4 MiB fp16 store per macroblock on
  separate HWDGE queues (SP loads, ACT stores).

Sharding: data-parallel across 8 cores along the batch dim; small
parameters replicated (per the problem's sharding hint).
"""

import math
import os
from contextlib import ExitStack

import numpy as np
import ml_dtypes

import concourse.bass as bass
import concourse.mybir as mybir
import concourse.tile as tile
from concourse import bacc
from concourse.masks import make_identity
from concourse.bass_utils import run_bass_kernel_spmd

F32 = mybir.dt.float32
F16 = mybir.dt.float16
BF16 = mybir.dt.bfloat16
AF = mybir.ActivationFunctionType
OP = mybir.AluOpType

N_CORES = 8
D = 4096
KSUB = 64
SUB = 128            # rows per subblock (one partition tile)
SPM = 4              # subblocks per macroblock
MACRO = SUB * SPM    # 512 rows
DCH = 128            # d-chunk width for PE transposes
NDCH = D // DCH      # 32
CCH = 512            # combine chunk width
NCCH = D // CCH      # 8

GP_CHUNKS = 3        # blend chunks per subblock routed to GPSIMD (of NCCH)
DVE_COPY_OF16 = 5    # of each macro's 16 transpose groups, this many copied by DVE
REGULAR_MM_TRANSPOSE = bool(int(os.environ.get("GOF_REGMM", "0")))

_PROGRAM_CACHE: dict = {}
LAST_RESULT = None


def _build_program(rows: int, num_steps: int, neg_inv: float, exp_bias: float,
                   neg_h: float):
    nmacro = rows // MACRO
    assert rows == nmacro * MACRO, f"rows {rows} not a multiple of {MACRO}"

    nc = bacc.Bacc("TRN2")
    x_d = nc.dram_tensor("x", [rows, D], F32, kind="ExternalInput")
    w_d = nc.dram_tensor("w", [D, KSUB], BF16, kind="ExternalInput")
    tgr_d = nc.dram_tensor("tgr", [128, D], BF16, kind="ExternalInput")
    nqt_d = nc.dram_tensor("nqt", [KSUB, 1], F32, kind="ExternalInput")
    abr_d = nc.dram_tensor("abr", [128, 2], BF16, kind="ExternalInput")
    out_d = nc.dram_tensor("out", [rows, D], F16, kind="ExternalOutput")

    with ExitStack() as ctx:
        tc = ctx.enter_context(tile.TileContext(nc))
        singles = ctx.enter_context(tc.tile_pool(name="singles", bufs=1))
        xpool = ctx.enter_context(tc.tile_pool(name="xp", bufs=9))
        outpool = ctx.enter_context(tc.tile_pool(name="op", bufs=2))
        xtpool = ctx.enter_context(tc.tile_pool(name="xtp", bufs=4))
        ttpool = ctx.enter_context(tc.tile_pool(name="ttp", bufs=2))
        xcpool = ctx.enter_context(tc.tile_pool(name="xcp", bufs=2))
        stkpool = ctx.enter_context(tc.tile_pool(name="stkp", bufs=2))
        smpool = ctx.enter_context(tc.tile_pool(name="smp", bufs=2))
        ptr = ctx.enter_context(tc.tile_pool(name="ptr", bufs=3, space="PSUM"))
        pq = ctx.enter_context(tc.tile_pool(name="pq", bufs=2, space="PSUM"))
        pab = ctx.enter_context(tc.tile_pool(name="pab", bufs=2, space="PSUM"))

        identu = singles.tile([128, 128], BF16)
        make_identity(nc, identu)
        w_sb = singles.tile([128, NDCH, KSUB], BF16)
        nc.sync.dma_start(out=w_sb, in_=w_d[:, :].rearrange("(j p) k -> p j k", p=128))
        tgr_sb = singles.tile([128, D], BF16)
        nc.sync.dma_start(out=tgr_sb, in_=tgr_d[:, :])
        nqt_sb = singles.tile([KSUB, 1], F32)
        nc.sync.dma_start(out=nqt_sb, in_=nqt_d[:, :])
        abr_sb = singles.tile([128, 2], BF16)
        nc.sync.dma_start(out=abr_sb, in_=abr_d[:, :])
        ebias_sb = singles.tile([128, 1], F32)
        nc.vector.memset(ebias_sb, exp_bias)

        def emit_front(m):
            """Load + PE transposes + bf16 projection + extraction + A/B."""
            r0 = m * MACRO
            xs = []
            xus = []
            for s in range(SPM):
                xin = xpool.tile([SUB, D], F32, tag="xin")
                nc.sync.dma_start(
                    out=xin, in_=x_d[r0 + s * SUB : r0 + (s + 1) * SUB, :])
                xs.append(xin)
                # stride-2 bf16 view: element 2d+1 is the high half of
                # x[d], i.e. the truncated-bf16 value of x[d]
                xus.append(xin.bitcast(BF16).rearrange(
                    "p (d two) -> p d two", two=2))

            q0T = pq.tile([KSUB, MACRO], F32, tag="q0T")
            if REGULAR_MM_TRANSPOSE:
                # Transposes as regular matmuls (identity moving) so the
                # PE HAM activity monitor sees them; fp32 PSUM out.
                for g in range(NDCH):
                    tp = ptr.tile([128, MACRO], F32, tag="tp")
                    for s in range(SPM):
                        nc.tensor.matmul(
                            tp[:, s * SUB : (s + 1) * SUB],
                            xus[s][:, g * DCH : (g + 1) * DCH, 1], identu,
                            start=True, stop=True)
                    xt = xtpool.tile([128, MACRO], BF16, tag="xt")
                    if (g % 16) < DVE_COPY_OF16:
                        nc.vector.tensor_copy(xt, tp)
                    else:
                        nc.scalar.copy(xt, tp)
                    nc.tensor.matmul(
                        q0T, w_sb[:, g, :], xt,
                        start=(g == 0), stop=(g == NDCH - 1))
            else:
                # PE transpose mode: bf16 PSUM, two d-chunks per bank.
                for g in range(NDCH // 2):
                    tp = ptr.tile([128, 2 * MACRO], BF16, tag="tp")
                    for jj in range(2):
                        j = 2 * g + jj
                        for s in range(SPM):
                            nc.tensor.transpose(
                                tp[:, jj * MACRO + s * SUB
                                   : jj * MACRO + (s + 1) * SUB],
                                xus[s][:, j * DCH : (j + 1) * DCH, 1], identu)
                    xt = xtpool.tile([128, 2 * MACRO], BF16, tag="xt")
                    if g < DVE_COPY_OF16:
                        nc.vector.tensor_copy(xt, tp)
                    else:
                        nc.scalar.copy(xt, tp)
                    nc.tensor.matmul(
                        q0T, w_sb[:, 2 * g, :], xt[:, 0:MACRO],
                        start=(g == 0), stop=False)
                    nc.tensor.matmul(
                        q0T, w_sb[:, 2 * g + 1, :], xt[:, MACRO : 2 * MACRO],
                        start=False, stop=(g == NDCH // 2 - 1))

            # stk rows 0..63 = (q0T - qT)^2 ; rows 64..127 = (q0T - qT)
            stk = stkpool.tile([128, MACRO], BF16, tag="stk")
            nc.scalar.activation(stk[0:KSUB, :], q0T, AF.Square,
                                 bias=nqt_sb, scale=1.0)
            nc.scalar.activation(stk[KSUB:128, :], q0T, AF.Identity,
                                 bias=nqt_sb, scale=1.0)
            # ab[:, 2s] = A_s, ab[:, 2s+1] = B2_s
            ab = pab.tile([128, 2 * SPM], F32, tag="ab")
            for s in range(SPM):
                nc.tensor.matmul(ab[:, 2 * s : 2 * s + 2],
                                 stk[:, s * SUB : (s + 1) * SUB],
                                 abr_sb[:, 0:2], start=True, stop=True)
            return {"xs": xs, "ab": ab, "r0": r0}

        def emit_iteration(st):
            """Per-row scalar recurrence (DVE + ACT exp) -> c, d."""
            ab = st["ab"]
            A = ab[:, 0 : 2 * SPM : 2]
            B2 = ab[:, 1 : 2 * SPM : 2]
            c = smpool.tile([128, SPM], F32, tag="c")
            nc.vector.memset(c, 1.0)
            t1 = smpool.tile([128, SPM], F32, tag="t1")
            alpha = smpool.tile([128, SPM], F32, tag="alpha")
            for _t in range(num_steps):
                nc.vector.tensor_tensor(t1, A, c, OP.mult)
                nc.vector.tensor_tensor(t1, t1, B2, OP.add)
                nc.vector.tensor_tensor(t1, t1, c, OP.mult)
                nc.scalar.activation(alpha, t1, AF.Exp,
                                     bias=ebias_sb, scale=neg_inv)
                nc.vector.tensor_tensor(t1, alpha, c, OP.mult)
                nc.vector.scalar_tensor_tensor(c, t1, neg_h, c, OP.mult, OP.add)
            d_t = smpool.tile([128, SPM], F32, tag="d")
            nc.vector.tensor_scalar(d_t, c, -1.0, 1.0, OP.mult, OP.add)
            st["c"] = c
            st["d_t"] = d_t

        def emit_blend_store(st):
            """out = c*x + (1-c)*tgt -> fp16, then store."""
            xs, c, d_t, r0 = st["xs"], st["c"], st["d_t"], st["r0"]
            n_dve = NCCH - GP_CHUNKS
            for s in range(SPM):
                cs = c[:, s : s + 1]
                ds = d_t[:, s : s + 1]
                ttmp = ttpool.tile([128, D], BF16, tag="ttmp")
                nc.vector.tensor_scalar(ttmp, tgr_sb, ds, None, OP.mult)
                out_t = outpool.tile([128, D], F16, tag="out")
                for h in range(n_dve):
                    sl = slice(h * CCH, (h + 1) * CCH)
                    nc.vector.scalar_tensor_tensor(
                        out_t[:, sl], xs[s][:, sl], cs, ttmp[:, sl],
                        OP.mult, OP.add)
                if GP_CHUNKS:
                    lo = n_dve * CCH
                    xc = xcpool.tile([128, GP_CHUNKS * CCH], BF16, tag="xc")
                    nc.scalar.activation(xc, xs[s][:, lo:D], AF.Copy,
                                         bias=0.0, scale=cs)
                    nc.gpsimd.tensor_tensor(out_t[:, lo:D], xc,
                                            ttmp[:, lo:D], OP.add)
                nc.scalar.dma_start(
                    out=out_d[r0 + s * SUB : r0 + (s + 1) * SUB, :],
                    in_=out_t)

        # Software-pipelined emission: macro m's blend is emitted after
        # macro m+1's PE-heavy front so the in-order PE stream never
        # waits on the serial gate recurrence.
        prev = None
        for m in range(nmacro):
            st = emit_front(m)
            if prev is not None:
                emit_blend_store(prev)
            emit_iteration(st)
            prev = st
        emit_blend_store(prev)

    if not nc.is_finalized():
        nc.finalize()
    return nc


def _get_program(rows, num_steps, neg_inv, exp_bias, neg_h):
    key = (rows, num_steps, neg_inv, exp_bias, neg_h,
           GP_CHUNKS, DVE_COPY_OF16, REGULAR_MM_TRANSPOSE)
    if key not in _PROGRAM_CACHE:
        _PROGRAM_CACHE[key] = _build_program(rows, num_steps, neg_inv,
                                             exp_bias, neg_h)
    return _PROGRAM_CACHE[key]


def kernel(x, manifold_mu, manifold_U, manifold_S, attractor_mu,
           log_step, sigma, num_steps):
    global LAST_RESULT
    x = np.ascontiguousarray(np.asarray(x, dtype=np.float32))
    mu = np.asarray(manifold_mu, dtype=np.float64)
    U = np.asarray(manifold_U, dtype=np.float64)
    S = np.asarray(manifold_S, dtype=np.float64)
    tgt = np.asarray(attractor_mu, dtype=np.float64)
    ls = float(np.asarray(log_step))
    sg = float(np.asarray(sigma))
    ns = int(np.asarray(num_steps))

    batch, dmodel = x.shape
    assert dmodel == D and batch % N_CORES == 0

    if ns <= 0:
        return x.copy()

    # Host-side parameter folding (O(D*K), trivial). qT/qmu/C use the
    # truncated-bf16 W so they are consistent with the device projection,
    # which feeds bf16(x)~truncation and bf16(W) into the matmul.
    W = U / (S + 1e-6)[None, :]
    W16 = W.astype(ml_dtypes.bfloat16)
    Wq = W16.astype(np.float64)
    qT = tgt @ Wq
    qmu = mu @ Wq
    wt = qT - qmu
    Cc = float(wt @ wt)
    inv = 1.0 / (float(KSUB) * 2.0 * sg * sg * 1.0)  # TEMPERATURE = 1.0
    step = min(max(math.exp(ls), 1e-3), 1.0)
    h = step / ns

    neg_inv = -inv
    exp_bias = -inv * Cc
    neg_h = -h

    rows = batch // N_CORES
    nc = _get_program(rows, ns, neg_inv, exp_bias, neg_h)

    abr = np.zeros((128, 2), ml_dtypes.bfloat16)
    abr[0:KSUB, 0] = 1.0
    abr[KSUB:128, 1] = (2.0 * wt).astype(ml_dtypes.bfloat16)
    tgr = np.ascontiguousarray(
        np.broadcast_to(tgt.astype(ml_dtypes.bfloat16)[None, :], (128, D)))
    common = {
        "w": np.ascontiguousarray(W16),
        "tgr": tgr,
        "nqt": np.ascontiguousarray((-qT).astype(np.float32)[:, None]),
        "abr": abr,
    }
    in_maps = [
        {"x": x[i * rows : (i + 1) * rows], **common} for i in range(N_CORES)
    ]

    trace = bool(int(os.environ.get("GOF_TRACE", "0")))
    res = run_bass_kernel_spmd(nc, in_maps, list(range(N_CORES)), trace=trace)
    LAST_RESULT = res
    out = np.concatenate([res.results[i]["out"] for i in range(N_CORES)],
                         axis=0)
    return out.astype(np.float32)


# revision 3
# speedup vs baseline: 1.1703x; 1.0874x over previous
"""Trainium2 Bass kernel for the GatedODEFlow problem.

Math: the reference iterates  a <- a + h*alpha(a) * (tgt - a)  where
alpha depends on a only through the low-rank projection (a - mu) @ U / S.
Since each step is a per-row convex blend toward the fixed vector tgt,
a_t = c_t * x + (1 - c_t) * tgt  for a per-row scalar c_t, and the
projection evolves affinely in c_t:

    proj_t = c_t * (x@W - tgt@W) + (tgt@W - mu@W)   with W = U / (S+1e-6)
    dist2_t = A * c_t^2 + B2 * c_t + C              (per-row A, B2; global C)
    alpha_t = exp(-dist2_t / (2*k*sigma^2))
    c_{t+1} = c_t * (1 - h * alpha_t),  c_0 = 1
    out = c_N * x + (1 - c_N) * tgt

So the device only needs ONE matmul q0 = x @ W per row plus a scalar
recurrence and a final fused blend.  The 2e-2 rel-err budget dwarfs bf16
rounding, so x lives on-chip in bf16 only (the SWDGE DMA path casts
fp32->bf16 in-flight at line rate) and the output is stored in bf16:
HBM traffic is 64 MiB read + 32 MiB write per core -- a ~270us roofline
at 358 GB/s -- and SBUF holds 2.5 macroblocks of prefetched x.

v3 engine layout (per 512-row macroblock):
- GPSIMD(SWDGE): cast-loads of x two macroblocks ahead; bf16 blend adds
  for the trailing GP_CHUNKS chunks of each subblock.
- PE: 128 transposes of bf16 x, 32 projection matmuls, 4 A/B matmuls.
- ACT: xc = c*x per subblock (per-partition `scale=` AP), most
  PSUM->SBUF copies of transposed groups, gate Square/Identity, exp.
- DVE: ttmp = (1-c)*tgt per subblock (4x-mode bf16 tensor_scalar),
  remaining copies, bf16 2x-mode blend adds out = xc + ttmp for the
  leading chunks, the scalar recurrence.
- SP: output stores (1 MiB bf16 per subblock), on their own HWDGE queue.

Emission is software-pipelined with consumption before production on
every engine queue (engines execute their queues in order): iteration m
emits blend/store of m-1, cast-loads of m+2, then the PE-heavy front
and gate recurrence of m.

Sharding: data-parallel across 8 cores along the batch dim; small
parameters replicated (per the problem's sharding hint).
"""

import math
import os
from contextlib import ExitStack

import numpy as np
import ml_dtypes

import concourse.bass as bass
import concourse.mybir as mybir
import concourse.tile as tile
from concourse import bacc
from concourse.masks import make_identity
from concourse.bass_utils import run_bass_kernel_spmd

F32 = mybir.dt.float32
F16 = mybir.dt.float16
BF16 = mybir.dt.bfloat16
AF = mybir.ActivationFunctionType
OP = mybir.AluOpType

N_CORES = 8
D = 4096
KSUB = 64
SUB = 128            # rows per subblock (one partition tile)
SPM = 4              # subblocks per macroblock
MACRO = SUB * SPM    # 512 rows
DCH = 128            # d-chunk width for PE transposes
NDCH = D // DCH      # 32
CCH = 512            # combine chunk width
NCCH = D // CCH      # 8

GP_CHUNKS = 3        # blend chunks per subblock routed to GPSIMD (of NCCH)
DVE_COPY_OF16 = 6    # of each macro's 16 transpose groups, this many copied by DVE

_PROGRAM_CACHE: dict = {}
LAST_RESULT = None


def _build_program(rows: int, num_steps: int, neg_inv: float, exp_bias: float,
                   neg_h: float):
    nmacro = rows // MACRO
    assert rows == nmacro * MACRO, f"rows {rows} not a multiple of {MACRO}"

    nc = bacc.Bacc("TRN2")
    x_d = nc.dram_tensor("x", [rows, D], F32, kind="ExternalInput")
    w_d = nc.dram_tensor("w", [D, KSUB], BF16, kind="ExternalInput")
    tgr_d = nc.dram_tensor("tgr", [128, D], BF16, kind="ExternalInput")
    nqt_d = nc.dram_tensor("nqt", [KSUB, 1], F32, kind="ExternalInput")
    abr_d = nc.dram_tensor("abr", [128, 2], BF16, kind="ExternalInput")
    out_d = nc.dram_tensor("out", [rows, D], BF16, kind="ExternalOutput")

    with ExitStack() as ctx:
        tc = ctx.enter_context(tile.TileContext(nc))
        singles = ctx.enter_context(tc.tile_pool(name="singles", bufs=1))
        xbpool = ctx.enter_context(tc.tile_pool(name="xb", bufs=14))
        outpool = ctx.enter_context(tc.tile_pool(name="op", bufs=3))
        xtpool = ctx.enter_context(tc.tile_pool(name="xtp", bufs=4))
        ttpool = ctx.enter_context(tc.tile_pool(name="ttp", bufs=2))
        xcpool = ctx.enter_context(tc.tile_pool(name="xcp", bufs=2))
        stkpool = ctx.enter_context(tc.tile_pool(name="stkp", bufs=2))
        smpool = ctx.enter_context(tc.tile_pool(name="smp", bufs=2))
        ptr = ctx.enter_context(tc.tile_pool(name="ptr", bufs=3, space="PSUM"))
        pq = ctx.enter_context(tc.tile_pool(name="pq", bufs=2, space="PSUM"))
        pab = ctx.enter_context(tc.tile_pool(name="pab", bufs=2, space="PSUM"))

        identu = singles.tile([128, 128], BF16)
        make_identity(nc, identu)
        w_sb = singles.tile([128, NDCH, KSUB], BF16)
        nc.sync.dma_start(out=w_sb, in_=w_d[:, :].rearrange("(j p) k -> p j k", p=128))
        tgr_sb = singles.tile([128, D], BF16)
        nc.sync.dma_start(out=tgr_sb, in_=tgr_d[:, :])
        nqt_sb = singles.tile([KSUB, 1], F32)
        nc.sync.dma_start(out=nqt_sb, in_=nqt_d[:, :])
        abr_sb = singles.tile([128, 2], BF16)
        nc.sync.dma_start(out=abr_sb, in_=abr_d[:, :])
        ebias_sb = singles.tile([128, 1], F32)
        nc.vector.memset(ebias_sb, exp_bias)

        def emit_loads(m):
            """SWDGE cast-loads fp32 -> bf16 for macro m."""
            r0 = m * MACRO
            xbs = []
            for s in range(SPM):
                xb = xbpool.tile([SUB, D], BF16, tag="xb")
                nc.gpsimd.dma_start(
                    out=xb, in_=x_d[r0 + s * SUB : r0 + (s + 1) * SUB, :])
                xbs.append(xb)
            return xbs

        def emit_front(m, xbs):
            """PE transposes + bf16 projection + extraction + A/B."""
            q0T = pq.tile([KSUB, MACRO], F32, tag="q0T")
            for g in range(NDCH // 2):
                tp = ptr.tile([128, 2 * MACRO], BF16, tag="tp")
                for jj in range(2):
                    j = 2 * g + jj
                    for s in range(SPM):
                        nc.tensor.transpose(
                            tp[:, jj * MACRO + s * SUB
                               : jj * MACRO + (s + 1) * SUB],
                            xbs[s][:, j * DCH : (j + 1) * DCH], identu)
                xt = xtpool.tile([128, 2 * MACRO], BF16, tag="xt")
                if g < DVE_COPY_OF16:
                    nc.vector.tensor_copy(xt, tp)
                else:
                    nc.scalar.copy(xt, tp)
                nc.tensor.matmul(
                    q0T, w_sb[:, 2 * g, :], xt[:, 0:MACRO],
                    start=(g == 0), stop=False)
                nc.tensor.matmul(
                    q0T, w_sb[:, 2 * g + 1, :], xt[:, MACRO : 2 * MACRO],
                    start=False, stop=(g == NDCH // 2 - 1))

            # stk rows 0..63 = (q0T - qT)^2 ; rows 64..127 = (q0T - qT)
            stk = stkpool.tile([128, MACRO], BF16, tag="stk")
            nc.scalar.activation(stk[0:KSUB, :], q0T, AF.Square,
                                 bias=nqt_sb, scale=1.0)
            nc.scalar.activation(stk[KSUB:128, :], q0T, AF.Identity,
                                 bias=nqt_sb, scale=1.0)
            # ab[:, 2s] = A_s, ab[:, 2s+1] = B2_s
            ab = pab.tile([128, 2 * SPM], F32, tag="ab")
            for s in range(SPM):
                nc.tensor.matmul(ab[:, 2 * s : 2 * s + 2],
                                 stk[:, s * SUB : (s + 1) * SUB],
                                 abr_sb[:, 0:2], start=True, stop=True)
            return {"xbs": xbs, "ab": ab, "r0": m * MACRO}

        def emit_iteration(st):
            """Per-row scalar recurrence (DVE + ACT exp) -> c, d."""
            ab = st["ab"]
            A = ab[:, 0 : 2 * SPM : 2]
            B2 = ab[:, 1 : 2 * SPM : 2]
            c = smpool.tile([128, SPM], F32, tag="c")
            nc.vector.memset(c, 1.0)
            t1 = smpool.tile([128, SPM], F32, tag="t1")
            alpha = smpool.tile([128, SPM], F32, tag="alpha")
            for _t in range(num_steps):
                nc.vector.tensor_tensor(t1, A, c, OP.mult)
                nc.vector.tensor_tensor(t1, t1, B2, OP.add)
                nc.vector.tensor_tensor(t1, t1, c, OP.mult)
                nc.scalar.activation(alpha, t1, AF.Exp,
                                     bias=ebias_sb, scale=neg_inv)
                nc.vector.tensor_tensor(t1, alpha, c, OP.mult)
                nc.vector.scalar_tensor_tensor(c, t1, neg_h, c, OP.mult, OP.add)
            d_t = smpool.tile([128, SPM], F32, tag="d")
            nc.vector.tensor_scalar(d_t, c, -1.0, 1.0, OP.mult, OP.add)
            st["c"] = c
            st["d_t"] = d_t

        def emit_blend_store(st):
            """out = c*x + (1-c)*tgt in bf16, then store from SP queue."""
            xbs, c, d_t, r0 = st["xbs"], st["c"], st["d_t"], st["r0"]
            n_dve = NCCH - GP_CHUNKS
            lo = n_dve * CCH
            xcs = []
            # all xc first so the ACT queue never stalls on DVE/GP adds
            for s in range(SPM):
                xc = xcpool.tile([128, D], BF16, tag="xc")
                nc.scalar.activation(xc, xbs[s], AF.Copy,
                                     bias=0.0, scale=c[:, s : s + 1])
                xcs.append(xc)
            for s in range(SPM):
                ttmp = ttpool.tile([128, D], BF16, tag="ttmp")
                nc.vector.tensor_scalar(ttmp, tgr_sb, d_t[:, s : s + 1],
                                        None, OP.mult)
                out_t = outpool.tile([128, D], BF16, tag="out")
                for h in range(n_dve):
                    sl = slice(h * CCH, (h + 1) * CCH)
                    nc.vector.tensor_tensor(out_t[:, sl], xcs[s][:, sl],
                                            ttmp[:, sl], OP.add)
                if GP_CHUNKS:
                    nc.gpsimd.tensor_tensor(out_t[:, lo:D], xcs[s][:, lo:D],
                                            ttmp[:, lo:D], OP.add)
                nc.sync.dma_start(
                    out=out_d[r0 + s * SUB : r0 + (s + 1) * SUB, :],
                    in_=out_t)

        # Software pipeline: consume (blend m-1) before produce (front m);
        # cast-loads run two macroblocks ahead.
        xbs_q = {m: emit_loads(m) for m in range(min(2, nmacro))}
        prev = None
        for m in range(nmacro):
            if prev is not None:
                emit_blend_store(prev)
            if m + 2 < nmacro:
                xbs_q[m + 2] = emit_loads(m + 2)
            st = emit_front(m, xbs_q.pop(m))
            emit_iteration(st)
            prev = st
        emit_blend_store(prev)

    if not nc.is_finalized():
        nc.finalize()
    return nc


def _get_program(rows, num_steps, neg_inv, exp_bias, neg_h):
    key = (rows, num_steps, neg_inv, exp_bias, neg_h,
           GP_CHUNKS, DVE_COPY_OF16)
    if key not in _PROGRAM_CACHE:
        _PROGRAM_CACHE[key] = _build_program(rows, num_steps, neg_inv,
                                             exp_bias, neg_h)
    return _PROGRAM_CACHE[key]


def kernel(x, manifold_mu, manifold_U, manifold_S, attractor_mu,
           log_step, sigma, num_steps):
    global LAST_RESULT
    x = np.ascontiguousarray(np.asarray(x, dtype=np.float32))
    mu = np.asarray(manifold_mu, dtype=np.float64)
    U = np.asarray(manifold_U, dtype=np.float64)
    S = np.asarray(manifold_S, dtype=np.float64)
    tgt = np.asarray(attractor_mu, dtype=np.float64)
    ls = float(np.asarray(log_step))
    sg = float(np.asarray(sigma))
    ns = int(np.asarray(num_steps))

    batch, dmodel = x.shape
    assert dmodel == D and batch % N_CORES == 0

    if ns <= 0:
        return x.copy()

    # Host-side parameter folding (O(D*K), trivial). qT/qmu/C use the
    # truncated-bf16 W so they are consistent with the device projection,
    # which feeds bf16(x) and bf16(W) into the matmul.
    W = U / (S + 1e-6)[None, :]
    W16 = W.astype(ml_dtypes.bfloat16)
    Wq = W16.astype(np.float64)
    qT = tgt @ Wq
    qmu = mu @ Wq
    wt = qT - qmu
    Cc = float(wt @ wt)
    inv = 1.0 / (float(KSUB) * 2.0 * sg * sg * 1.0)  # TEMPERATURE = 1.0
    step = min(max(math.exp(ls), 1e-3), 1.0)
    h = step / ns

    neg_inv = -inv
    exp_bias = -inv * Cc
    neg_h = -h

    rows = batch // N_CORES
    nc = _get_program(rows, ns, neg_inv, exp_bias, neg_h)

    abr = np.zeros((128, 2), ml_dtypes.bfloat16)
    abr[0:KSUB, 0] = 1.0
    abr[KSUB:128, 1] = (2.0 * wt).astype(ml_dtypes.bfloat16)
    tgr = np.ascontiguousarray(
        np.broadcast_to(tgt.astype(ml_dtypes.bfloat16)[None, :], (128, D)))
    common = {
        "w": np.ascontiguousarray(W16),
        "tgr": tgr,
        "nqt": np.ascontiguousarray((-qT).astype(np.float32)[:, None]),
        "abr": abr,
    }
    in_maps = [
        {"x": x[i * rows : (i + 1) * rows], **common} for i in range(N_CORES)
    ]

    trace = bool(int(os.environ.get("GOF_TRACE", "0")))
    res = run_bass_kernel_spmd(nc, in_maps, list(range(N_CORES)), trace=trace)
    LAST_RESULT = res
    out = np.concatenate([res.results[i]["out"] for i in range(N_CORES)],
                         axis=0)
    return out.astype(np.float32)


# revision 8
# speedup vs baseline: 1.5592x; 1.3323x over previous
"""Trainium2 Bass kernel for the GatedODEFlow problem.

Math: the reference iterates  a <- a + h*alpha(a) * (tgt - a)  where
alpha depends on a only through the low-rank projection (a - mu) @ U / S.
Since each step is a per-row convex blend toward the fixed vector tgt,
a_t = c_t * x + (1 - c_t) * tgt  for a per-row scalar c_t, and the
projection evolves affinely in c_t:

    proj_t = c_t * (x@W - tgt@W) + (tgt@W - mu@W)   with W = U / (S+1e-6)
    dist2_t = A * c_t^2 + B2 * c_t + C              (per-row A, B2; global C)
    alpha_t = exp(-dist2_t / (2*k*sigma^2))
    c_{t+1} = c_t * (1 - h * alpha_t),  c_0 = 1
    out = c_N * x + (1 - c_N) * tgt

So the device only needs ONE matmul q0 = x @ W per row plus a scalar
recurrence and a final fused blend.  The 2e-2 rel-err budget dwarfs bf16
rounding, so x lives on-chip in bf16 only (the SWDGE DMA path casts
fp32->bf16 in-flight at line rate) and the output is stored in bf16:
HBM traffic is 64 MiB read + 32 MiB write per core -- a ~270us roofline
at 358 GB/s -- and SBUF holds 2.5 macroblocks of prefetched x.

v4 engine layout (per 512-row macroblock):
- GPSIMD(SWDGE): cast-loads of x two macroblocks ahead.  Nothing else:
  concurrent GPSIMD tensor ops contend for the SBUF port shared with
  the Vector engine and halve DVE 2-src throughput.
- PE: 128 transposes of bf16 x, 32 projection matmuls, 4 A/B matmuls.
- ACT: most PSUM->SBUF copies of transposed groups, gate
  Square/Identity extraction, exp.
- DVE: per subblock ttmp = (1-c)*tgt (4x-mode bf16 tensor_scalar with
  per-partition scalar) and one full-row in-place blend
  xb <- c*xb + ttmp; a few copies; the scalar recurrence.
- SP: output stores (1 MiB bf16 per subblock), on their own HWDGE queue.

Emission is software-pipelined with consumption before production on
every engine queue (engines execute their queues in order): iteration m
emits blend/store of m-1, cast-loads of m+2, then the PE-heavy front
and gate recurrence of m.

Sharding: data-parallel across 8 cores along the batch dim; small
parameters replicated (per the problem's sharding hint).
"""

import math
import os
from contextlib import ExitStack

import numpy as np
import ml_dtypes

import concourse.bass as bass
import concourse.mybir as mybir
import concourse.tile as tile
from concourse import bacc
from concourse.masks import make_identity
from concourse.bass_utils import run_bass_kernel_spmd

F32 = mybir.dt.float32
F16 = mybir.dt.float16
BF16 = mybir.dt.bfloat16
AF = mybir.ActivationFunctionType
OP = mybir.AluOpType

N_CORES = 8
D = 4096
KSUB = 64
SUB = 128            # rows per subblock (one partition tile)
SPM = 4              # subblocks per macroblock
MACRO = SUB * SPM    # 512 rows
DCH = 128            # d-chunk width for PE transposes
NDCH = D // DCH      # 32
CCH = 512            # combine chunk width
NCCH = D // CCH      # 8

GP_CHUNKS = 0        # blend chunks per subblock routed to GPSIMD (of NCCH):
                     # concurrent GPSIMD tensor ops contend for the shared
                     # SBUF port and halve DVE 2-src throughput, so GPSIMD
                     # only runs the SWDGE cast-loads now.
DVE_COPY_OF16 = 2    # of each macro's 16 transpose groups, this many copied by DVE
BLEND_MODE = os.environ.get("GOF_BLEND", "stt")  # 'stt' or 'tt'

_PROGRAM_CACHE: dict = {}
LAST_RESULT = None


def _build_program(rows: int, num_steps: int, neg_inv: float, exp_bias: float,
                   neg_h: float):
    nmacro = rows // MACRO
    assert rows == nmacro * MACRO, f"rows {rows} not a multiple of {MACRO}"

    nc = bacc.Bacc("TRN2")
    x_d = nc.dram_tensor("x", [rows, D], F32, kind="ExternalInput")
    w_d = nc.dram_tensor("w", [D, KSUB], BF16, kind="ExternalInput")
    tgr_d = nc.dram_tensor("tgr", [128, D], BF16, kind="ExternalInput")
    nqt_d = nc.dram_tensor("nqt", [KSUB, 1], F32, kind="ExternalInput")
    abr_d = nc.dram_tensor("abr", [128, 2], BF16, kind="ExternalInput")
    out_d = nc.dram_tensor("out", [rows, D], BF16, kind="ExternalOutput")

    with ExitStack() as ctx:
        tc = ctx.enter_context(tile.TileContext(nc))
        singles = ctx.enter_context(tc.tile_pool(name="singles", bufs=1))
        xbpool = ctx.enter_context(tc.tile_pool(name="xb", bufs=16))
        xtpool = ctx.enter_context(tc.tile_pool(name="xtp", bufs=4))
        ttpool = ctx.enter_context(tc.tile_pool(name="ttp", bufs=2))
        stkpool = ctx.enter_context(tc.tile_pool(name="stkp", bufs=2))
        smpool = ctx.enter_context(tc.tile_pool(name="smp", bufs=2))
        ptr = ctx.enter_context(tc.tile_pool(name="ptr", bufs=3, space="PSUM"))
        pq = ctx.enter_context(tc.tile_pool(name="pq", bufs=2, space="PSUM"))
        pab = ctx.enter_context(tc.tile_pool(name="pab", bufs=2, space="PSUM"))

        identu = singles.tile([128, 128], BF16)
        make_identity(nc, identu)
        w_sb = singles.tile([128, NDCH, KSUB], BF16)
        nc.sync.dma_start(out=w_sb, in_=w_d[:, :].rearrange("(j p) k -> p j k", p=128))
        tgr_sb = singles.tile([128, D], BF16)
        nc.sync.dma_start(out=tgr_sb, in_=tgr_d[:, :])
        nqt_sb = singles.tile([KSUB, 1], F32)
        nc.sync.dma_start(out=nqt_sb, in_=nqt_d[:, :])
        abr_sb = singles.tile([128, 2], BF16)
        nc.sync.dma_start(out=abr_sb, in_=abr_d[:, :])
        ebias_sb = singles.tile([128, 1], F32)
        nc.vector.memset(ebias_sb, exp_bias)

        def emit_loads(m):
            """SWDGE cast-loads fp32 -> bf16 for macro m."""
            r0 = m * MACRO
            xbs = []
            for s in range(SPM):
                xb = xbpool.tile([SUB, D], BF16, tag="xb")
                nc.gpsimd.dma_start(
                    out=xb, in_=x_d[r0 + s * SUB : r0 + (s + 1) * SUB, :])
                xbs.append(xb)
            return xbs

        def emit_front(m, xbs):
            """PE transposes + bf16 projection + extraction + A/B."""
            q0T = pq.tile([KSUB, MACRO], F32, tag="q0T")
            for g in range(NDCH // 2):
                tp = ptr.tile([128, 2 * MACRO], BF16, tag="tp")
                for jj in range(2):
                    j = 2 * g + jj
                    for s in range(SPM):
                        nc.tensor.transpose(
                            tp[:, jj * MACRO + s * SUB
                               : jj * MACRO + (s + 1) * SUB],
                            xbs[s][:, j * DCH : (j + 1) * DCH], identu)
                xt = xtpool.tile([128, 2 * MACRO], BF16, tag="xt")
                if g < DVE_COPY_OF16:
                    nc.vector.tensor_copy(xt, tp)
                else:
                    nc.scalar.copy(xt, tp)
                nc.tensor.matmul(
                    q0T, w_sb[:, 2 * g, :], xt[:, 0:MACRO],
                    start=(g == 0), stop=False)
                nc.tensor.matmul(
                    q0T, w_sb[:, 2 * g + 1, :], xt[:, MACRO : 2 * MACRO],
                    start=False, stop=(g == NDCH // 2 - 1))

            # stk rows 0..63 = (q0T - qT)^2 ; rows 64..127 = (q0T - qT)
            stk = stkpool.tile([128, MACRO], BF16, tag="stk")
            nc.scalar.activation(stk[0:KSUB, :], q0T, AF.Square,
                                 bias=nqt_sb, scale=1.0)
            nc.scalar.activation(stk[KSUB:128, :], q0T, AF.Identity,
                                 bias=nqt_sb, scale=1.0)
            # ab[:, 2s] = A_s, ab[:, 2s+1] = B2_s
            ab = pab.tile([128, 2 * SPM], F32, tag="ab")
            for s in range(SPM):
                nc.tensor.matmul(ab[:, 2 * s : 2 * s + 2],
                                 stk[:, s * SUB : (s + 1) * SUB],
                                 abr_sb[:, 0:2], start=True, stop=True)
            return {"xbs": xbs, "ab": ab, "r0": m * MACRO}

        def emit_iteration(st):
            """Per-row scalar recurrence (DVE + ACT exp) -> c, d."""
            ab = st["ab"]
            A = ab[:, 0 : 2 * SPM : 2]
            B2 = ab[:, 1 : 2 * SPM : 2]
            c = smpool.tile([128, SPM], F32, tag="c")
            nc.vector.memset(c, 1.0)
            t1 = smpool.tile([128, SPM], F32, tag="t1")
            alpha = smpool.tile([128, SPM], F32, tag="alpha")
            for _t in range(num_steps):
                nc.vector.tensor_tensor(t1, A, c, OP.mult)
                nc.vector.tensor_tensor(t1, t1, B2, OP.add)
                nc.vector.tensor_tensor(t1, t1, c, OP.mult)
                nc.scalar.activation(alpha, t1, AF.Exp,
                                     bias=ebias_sb, scale=neg_inv)
                nc.vector.tensor_tensor(t1, alpha, c, OP.mult)
                nc.vector.scalar_tensor_tensor(c, t1, neg_h, c, OP.mult, OP.add)
            d_t = smpool.tile([128, SPM], F32, tag="d")
            nc.vector.tensor_scalar(d_t, c, -1.0, 1.0, OP.mult, OP.add)
            st["c"] = c
            st["d_t"] = d_t

        def emit_blend_store(st):
            """xb <- c*xb + (1-c)*tgt in place (bf16), then store from SP."""
            xbs, c, d_t, r0 = st["xbs"], st["c"], st["d_t"], st["r0"]
            for s in range(SPM):
                cs = c[:, s : s + 1]
                ttmp = ttpool.tile([128, D], BF16, tag="ttmp")
                nc.vector.tensor_scalar(ttmp, tgr_sb, d_t[:, s : s + 1],
                                        None, OP.mult)
                if BLEND_MODE == "stt":
                    nc.vector.scalar_tensor_tensor(
                        xbs[s], xbs[s], cs, ttmp, OP.mult, OP.add)
                else:
                    nc.vector.tensor_scalar(xbs[s], xbs[s], cs, None, OP.mult)
                    nc.vector.tensor_tensor(xbs[s], xbs[s], ttmp, OP.add)
                nc.sync.dma_start(
                    out=out_d[r0 + s * SUB : r0 + (s + 1) * SUB, :],
                    in_=xbs[s])

        # Software pipeline: consume (blend m-1) before produce (front m);
        # cast-loads run two macroblocks ahead.
        xbs_q = {m: emit_loads(m) for m in range(min(2, nmacro))}
        prev = None
        for m in range(nmacro):
            if prev is not None:
                emit_blend_store(prev)
            if m + 2 < nmacro:
                xbs_q[m + 2] = emit_loads(m + 2)
            st = emit_front(m, xbs_q.pop(m))
            emit_iteration(st)
            prev = st
        emit_blend_store(prev)

    if not nc.is_finalized():
        nc.finalize()
    return nc


def _get_program(rows, num_steps, neg_inv, exp_bias, neg_h):
    key = (rows, num_steps, neg_inv, exp_bias, neg_h,
           GP_CHUNKS, DVE_COPY_OF16, BLEND_MODE)
    if key not in _PROGRAM_CACHE:
        _PROGRAM_CACHE[key] = _build_program(rows, num_steps, neg_inv,
                                             exp_bias, neg_h)
    return _PROGRAM_CACHE[key]


def kernel(x, manifold_mu, manifold_U, manifold_S, attractor_mu,
           log_step, sigma, num_steps):
    global LAST_RESULT
    x = np.ascontiguousarray(np.asarray(x, dtype=np.float32))
    mu = np.asarray(manifold_mu, dtype=np.float64)
    U = np.asarray(manifold_U, dtype=np.float64)
    S = np.asarray(manifold_S, dtype=np.float64)
    tgt = np.asarray(attractor_mu, dtype=np.float64)
    ls = float(np.asarray(log_step))
    sg = float(np.asarray(sigma))
    ns = int(np.asarray(num_steps))

    batch, dmodel = x.shape
    assert dmodel == D and batch % N_CORES == 0

    if ns <= 0:
        return x.copy()

    # Host-side parameter folding (O(D*K), trivial). qT/qmu/C use the
    # truncated-bf16 W so they are consistent with the device projection,
    # which feeds bf16(x) and bf16(W) into the matmul.
    W = U / (S + 1e-6)[None, :]
    W16 = W.astype(ml_dtypes.bfloat16)
    Wq = W16.astype(np.float64)
    qT = tgt @ Wq
    qmu = mu @ Wq
    wt = qT - qmu
    Cc = float(wt @ wt)
    inv = 1.0 / (float(KSUB) * 2.0 * sg * sg * 1.0)  # TEMPERATURE = 1.0
    step = min(max(math.exp(ls), 1e-3), 1.0)
    h = step / ns

    neg_inv = -inv
    exp_bias = -inv * Cc
    neg_h = -h

    rows = batch // N_CORES
    nc = _get_program(rows, ns, neg_inv, exp_bias, neg_h)

    abr = np.zeros((128, 2), ml_dtypes.bfloat16)
    abr[0:KSUB, 0] = 1.0
    abr[KSUB:128, 1] = (2.0 * wt).astype(ml_dtypes.bfloat16)
    tgr = np.ascontiguousarray(
        np.broadcast_to(tgt.astype(ml_dtypes.bfloat16)[None, :], (128, D)))
    common = {
        "w": np.ascontiguousarray(W16),
        "tgr": tgr,
        "nqt": np.ascontiguousarray((-qT).astype(np.float32)[:, None]),
        "abr": abr,
    }
    in_maps = [
        {"x": x[i * rows : (i + 1) * rows], **common} for i in range(N_CORES)
    ]

    trace = bool(int(os.environ.get("GOF_TRACE", "0")))
    res = run_bass_kernel_spmd(nc, in_maps, list(range(N_CORES)), trace=trace)
    LAST_RESULT = res
    out = np.concatenate([res.results[i]["out"] for i in range(N_CORES)],
                         axis=0)
    return out.astype(np.float32)
